# revision 15
# baseline (speedup 1.0000x reference)
"""CPC InfoNCE loss kernel for Trainium2 (8 NeuronCores, data-parallel rows).

v4: fp8 DoubleRow similarity matmuls with the count-mask folded into PSUM,
exp split across ACT and DVE (Schraudolph bit-trick), no dense count DMA.

Per core (rows sharded across cores, 3 horizons x 8 blocks of 128 rows):
  - Host normalizes the pool table az = normalize(z_seq.reshape(BT, D)),
    quantizes 4*az to fp8-e4m3 once (AZQ, transposed), gathers the positives
    from the same quantized table (AZPQ), and packs predictor weights (bf16).
    Instead of dense bf16 counts it sends LNC [row, pool] fp8 with
    LNC = ln(multiplicity) * 2*tau*||u_row||  at sampled pool entries
    (positive included) and -240 elsewhere.
  - PE computes U^T = W @ Z_anchor^T in bf16 into PSUM; DVE casts to fp8.
    Norms ||u||^2 / positive logits via fp8 squares + ones-matmuls (PSUM
    accumulated), rs = 1/(4*tau*||u||) via DVE reciprocal + ACT sqrt.
  - Per 128-row block and 512-col slice PE issues TWO fp8 DoubleRow matmuls
    into one PSUM group: S = U_blk @ AZQ^T (K=256) plus IDM @ LNC (paired
    identity, stride-0 rhs) which adds 2*LNC. After the per-row exp scale
    rs, the PSUM value is logit + ln(cnt) (or ~-27 at unsampled entries, so
    exp gives 0 and the row sum needs no mask).
  - exp+reduce: ACT blocks use activation(Exp, scale=rs, accum_out) giving
    sum_j cnt_j e^j directly; Schraudolph blocks compute a piecewise-linear
    2^t on DVE (tensor_scalar f32->uint16 with scale rs*128*log2(e), bitcast
    to bf16) and reduce with a 4x-mode tensor_scalar accumulate. The exact
    E[(1+f)2^-f] = 0.5/ln(2)^2 bias of the PWL exp is subtracted on the host
    (ln R shifts by 0.0398844) for those blocks.
  - loss = ln(R) - s_pos per row; host averages with horizon weights.
"""

import sys

sys.path.insert(0, "/opt/trn_rl_repo")

import math
import os

import ml_dtypes
import numpy as np

import concourse.bass as bass
import concourse.tile as tile
from concourse import bacc
from concourse import mybir
from concourse.bass_utils import run_bass_kernel_spmd

# Problem constants (hardcoded per contract)
B, T, D = 16, 512, 256
BT = B * T  # 8192 pool entries
HORIZONS = (1, 5, 21)
H = len(HORIZONS)
N_NEG = 128
TAU = 0.07
N_CORES = 8

P = 128
NROW = 1024  # padded rows per core per horizon
NBLK = NROW // P  # 8
NCOL = H * NBLK  # 24 row-blocks per core
QCOL = 2048  # exp tile width (4 PSUM banks)
NQ = BT // QCOL  # 4
QCOL2 = 1024  # split-stream exp tile width (2 PSUM banks)
NQ2 = BT // QCOL2  # 8

AZ_SCALE = 4.0  # fp8 pre-scale on the pool table
# "masked" value: adds 2*(-144) to PSUM -> exp arg ~ -65 +- 12, so exp -> 0
# while the Schraudolph uint16 bits stay positive (arg > -88).
LNC_NEG = -144.0
LOG2E_128 = 128 * 1.4426950408889634  # Schraudolph scale
SCHRAUDOLPH_C = 16256.5  # 127<<7 | +0.5 for trunc->round
LN_PWL_BIAS = float(np.log(0.5 / np.log(2.0) ** 2))  # 0.0398844...

BF16 = mybir.dt.bfloat16
F32 = mybir.dt.float32
FP8 = mybir.dt.float8e4
U16 = mybir.dt.uint16

# block columns whose exp runs on DVE via the Schraudolph trick
N_DVE_EXP = int(os.environ.get("KERNEL_DVE_EXP", "9"))
_RB_PATTERNS = {
    0: (), 1: (5,), 2: (2, 6), 3: (2, 5, 7), 4: (1, 3, 5, 7),
    5: (1, 3, 5, 6, 7), 6: (1, 2, 3, 5, 6, 7), 7: (1, 2, 3, 4, 5, 6, 7),
    8: tuple(range(8)),
}


def _dve_exp_cols():
    base, rem = divmod(N_DVE_EXP, H)
    cols = set()
    for i in range(H):
        cnt = base + (1 if i < rem else 0)
        for rb in _RB_PATTERNS[min(cnt, 8)]:
            cols.add(i * NBLK + rb)
    return frozenset(cols)


DVE_COLS = _dve_exp_cols()


def _split_multiwait_drains(nc):
    """This walrus build accepts only one sync-wait command per TPB_CTRL
    instruction; TileContext's exit drain carries one wait per live proc.
    Split the extras into preceding single-wait drains."""
    for f in nc.m.functions:
        for bb in f.blocks:
            new_list = []
            for inst in bb.instructions:
                si = inst.sync_info
                if si is not None and si.on_wait and len(si.on_wait) > 1:
                    waits = list(si.on_wait)
                    for j, w in enumerate(waits[:-1]):
                        d = mybir.InstDrain(
                            name=f"{inst.name}-w{j}", ins=[], outs=[]
                        )
                        d.engine = inst.engine
                        d.sync_info = mybir.SyncInfo(on_wait=[w], on_update=[])
                        nc.register_instruction(d)
                        new_list.append(d)
                    si.on_wait = [waits[-1]]
                    inst.sync_info = si
                new_list.append(inst)
            bb.instructions[:] = new_list


def build_program(reps=1):
    reps = int(os.environ.get("KERNEL_REPS", reps))
    nc = bacc.Bacc(
        "TRN2", target_bir_lowering=False, debug=False, num_devices=N_CORES
    )

    azq_d = nc.declare_dram_parameter("azq", [P, 2, BT], FP8, isOutput=False)
    zat_d = nc.declare_dram_parameter("zat", [P, H * 2, NROW], BF16, isOutput=False)
    azpq_d = nc.declare_dram_parameter("azpq", [P, H * 2, NROW], FP8, isOutput=False)
    pt_d = nc.declare_dram_parameter("pt", [P, H * 4, P], BF16, isOutput=False)
    idm_d = nc.declare_dram_parameter("idm", [P, 2, P], FP8, isOutput=False)
    lnc_d = nc.declare_dram_parameter("lnc", [P, NCOL, BT], FP8, isOutput=False)
    loss_d = nc.declare_dram_parameter("loss", [P, NCOL], F32, isOutput=True)

    from contextlib import ExitStack, nullcontext

    MULT = mybir.AluOpType.mult
    ADD = mybir.AluOpType.add

    with tile.TileContext(nc) as tc, ExitStack() as ctx:
        singles = ctx.enter_context(tc.tile_pool(name="singles", bufs=1))
        utq_pool = ctx.enter_context(tc.tile_pool(name="utq", bufs=4))
        junk_pool = ctx.enter_context(tc.tile_pool(name="junk", bufs=2))
        c_pool = ctx.enter_context(tc.tile_pool(name="c", bufs=3))
        e_pool = ctx.enter_context(tc.tile_pool(name="e", bufs=2))
        racc_pool = ctx.enter_context(tc.tile_pool(name="racc", bufs=2))
        small = ctx.enter_context(tc.tile_pool(name="small", bufs=2))
        # two independent 2-bank PSUM streams so ACT-exp blocks and
        # DVE-Schraudolph blocks pipeline concurrently
        psum_a = ctx.enter_context(tc.tile_pool(name="psum_a", bufs=2, space="PSUM"))
        psum_d = ctx.enter_context(tc.tile_pool(name="psum_d", bufs=2, space="PSUM"))

        # ---- preload constants -------------------------------------------
        azq_sb = singles.tile([P, 2, BT], FP8)
        nc.sync.dma_start(out=azq_sb[:], in_=azq_d[:])
        zat_sb = singles.tile([P, H * 2, NROW], BF16)
        nc.sync.dma_start(out=zat_sb[:], in_=zat_d[:])
        azpq_sb = singles.tile([P, H * 2, NROW], FP8)
        nc.sync.dma_start(out=azpq_sb[:], in_=azpq_d[:])
        pt_sb = singles.tile([P, H * 4, P], BF16)
        nc.sync.dma_start(out=pt_sb[:], in_=pt_d[:])
        idm_sb = singles.tile([P, 2, P], FP8)
        nc.sync.dma_start(out=idm_sb[:], in_=idm_d[:])

        ones_sb = singles.tile([P, 1], BF16)
        nc.vector.memset(ones_sb[:], 1.0)
        one1_sb = singles.tile([1, 1], F32)
        nc.vector.memset(one1_sb[:], 1.0)

        loss_sb = singles.tile([P, NCOL], F32)
        rsum_sb = singles.tile([P, NCOL], F32)
        rsT_sb = singles.tile([P, NCOL], F32)
        rsT2_sb = singles.tile([P, NCOL], F32)  # rs * 128*log2(e)
        spT_sb = singles.tile([P, NCOL], F32)
        edump_sb = singles.tile([P, QCOL], BF16)  # ACT exp dummy out

        def emit_u_phase(i):
            """U^T = W @ Z_anchor^T (bf16 -> PSUM -> fp8), norms, positive
            logits, and the transposed per-block scale/pos columns."""
            utq = utq_pool.tile([P, 2, NROW], FP8, tag="utq")
            t_n = psum_d.tile([P, 1024], F32, tag="pd")
            t_p = psum_d.tile([P, 1024], F32, tag="pd")
            for mc in range(2):
                t_u = psum_a.tile([P, 1024], F32, tag="pa")
                for nh in range(2):
                    nsl = slice(nh * 512, (nh + 1) * 512)
                    pu = t_u[:, nsl]
                    for kc in range(2):
                        nc.tensor.matmul(
                            pu,
                            pt_sb[:, i * 4 + kc * 2 + mc, :],
                            zat_sb[:, i * 2 + kc, nsl],
                            start=(kc == 0),
                            stop=(kc == 1),
                        )
                    nc.vector.tensor_copy(out=utq[:, mc, nsl], in_=pu)
                # norms and positive logits from the fp8-rounded values
                usq = junk_pool.tile([P, NROW], BF16, tag="usq")
                nc.vector.tensor_tensor(
                    usq[:], utq[:, mc, :], utq[:, mc, :], MULT
                )
                upr = junk_pool.tile([P, NROW], BF16, tag="upr")
                nc.vector.tensor_tensor(
                    upr[:], utq[:, mc, :], azpq_sb[:, i * 2 + mc, :], MULT
                )
                # column sums via ones-matmuls, accumulated in PSUM
                for nh in range(2):
                    nsl = slice(nh * 512, (nh + 1) * 512)
                    nc.tensor.matmul(
                        t_n[0:1, nsl], ones_sb[:], usq[:, nsl],
                        start=(mc == 0), stop=(mc == 1),
                    )
                    nc.tensor.matmul(
                        t_p[0:1, nsl], ones_sb[:], upr[:, nsl],
                        start=(mc == 0), stop=(mc == 1),
                    )
            # rs = 1/(4*tau*||u||) = sqrt(1/(16 tau^2 ||u||^2))
            recip = small.tile([1, NROW], F32, tag="recip")
            nc.vector.reciprocal(out=recip[:], in_=t_n[0:1, :])
            rs_flat = small.tile([1, NROW], F32, tag="rsflat")
            nc.scalar.activation(
                out=rs_flat[:], in_=recip[:],
                func=mybir.ActivationFunctionType.Sqrt,
                scale=float(1.0 / (16.0 * TAU * TAU)),
            )
            # sp = raw4_pos_dot * rs  (the positive logit)
            sp_flat = small.tile([1, NROW], F32, tag="spflat")
            nc.vector.tensor_tensor(
                sp_flat[:], t_p[0:1, :], rs_flat[:], MULT
            )
            # transpose the per-row scalars into per-block columns
            t_t = psum_a.tile([P, 1024], F32, tag="pa")
            for rb in range(NBLK):
                nc.tensor.matmul(
                    t_t[:, rb:rb + 1],
                    rs_flat[0:1, rb * P:(rb + 1) * P],
                    one1_sb[:], start=True, stop=True,
                )
                nc.tensor.matmul(
                    t_t[:, NBLK + rb:NBLK + rb + 1],
                    sp_flat[0:1, rb * P:(rb + 1) * P],
                    one1_sb[:], start=True, stop=True,
                )
            csl = slice(i * NBLK, (i + 1) * NBLK)
            nc.vector.tensor_copy(out=rsT_sb[:, csl], in_=t_t[:, 0:NBLK])
            nc.vector.tensor_copy(
                out=spT_sb[:, csl], in_=t_t[:, NBLK:2 * NBLK]
            )
            nc.vector.tensor_scalar(
                out=rsT2_sb[:, csl], in0=rsT_sb[:, csl],
                scalar1=float(LOG2E_128), scalar2=None, op0=MULT,
            )
            return utq

        def emit_s_block(i, rb, utq):
            """fp8 DR S matmuls + lncnt add -> exp -> row-sum."""
            col = i * NBLK + rb
            lnc_sb = c_pool.tile([P, 1, BT], FP8, tag="c")
            nc.sync.dma_start(out=lnc_sb[:], in_=lnc_d[:, col:col + 1, :])
            on_dve = col in DVE_COLS
            if on_dve:
                eu = e_pool.tile([P, BT], U16, tag="e")
            else:
                racc = racc_pool.tile([P, NQ2], F32, tag="racc")
            for q in range(NQ2):
                if on_dve:
                    ps = psum_d.tile([P, QCOL2], F32, tag="pd")
                else:
                    ps = psum_a.tile([P, QCOL2], F32, tag="pa")
                for sq in range(QCOL2 // 512):
                    c0 = q * QCOL2 + sq * 512
                    sl = ps[:, sq * 512:(sq + 1) * 512]
                    nc.tensor.matmul(
                        sl,
                        utq[:, 0:2, rb * P:(rb + 1) * P],
                        azq_sb[:, 0:2, c0:c0 + 512],
                        start=True, stop=False,
                        perf_mode=mybir.MatmulPerfMode.DoubleRow,
                    )
                    nc.tensor.matmul(
                        sl,
                        idm_sb[:],
                        lnc_sb[:, 0:1, c0:c0 + 512].broadcast_to([P, 2, 512]),
                        start=False, stop=True,
                        perf_mode=mybir.MatmulPerfMode.DoubleRow,
                    )
                if on_dve:
                    # e~ = 2^(x*log2e) via PWL bit-trick: uint16 bits of bf16
                    nc.vector.tensor_scalar(
                        out=eu[:, q * QCOL2:(q + 1) * QCOL2], in0=ps[:],
                        scalar1=rsT2_sb[:, col:col + 1],
                        scalar2=float(SCHRAUDOLPH_C),
                        op0=MULT, op1=ADD,
                    )
                else:
                    nc.scalar.activation(
                        out=edump_sb[:, 0:QCOL2], in_=ps[:],
                        func=mybir.ActivationFunctionType.Exp,
                        scale=rsT_sb[:, col:col + 1],
                        accum_out=racc[:, q:q + 1],
                    )
            if on_dve:
                ebf = eu[:].bitcast(BF16)
                nc.vector.tensor_scalar(
                    out=ebf, in0=ebf, scalar1=1.0, scalar2=0.0,
                    op0=MULT, op1=ADD,
                    accum_out=rsum_sb[:, col:col + 1],
                )
            else:
                nc.vector.tensor_scalar(
                    out=racc[:], in0=racc[:], scalar1=1.0, scalar2=0.0,
                    op0=MULT, op1=ADD,
                    accum_out=rsum_sb[:, col:col + 1],
                )

        unroll = int(os.environ.get("KERNEL_UNROLL", "1"))
        jit_u = int(os.environ.get("KERNEL_JIT_U", "0"))
        loop_cm = tc.For_i(0, reps, 1) if reps > 1 else nullcontext()
        with loop_cm:
            for _ in range(unroll):
                if jit_u:
                    # just-in-time U phases: emit U(i+1) two blocks into
                    # horizon i's S stream so the exp pipeline never drains
                    utqs = [emit_u_phase(0), None, None]
                    for i in range(H):
                        for rb in range(NBLK):
                            emit_s_block(i, rb, utqs[i])
                            if rb == 1 and i + 1 < H:
                                utqs[i + 1] = emit_u_phase(i + 1)
                else:
                    utqs = [emit_u_phase(i) for i in range(H)]
                    for i in range(H):
                        for rb in range(NBLK):
                            emit_s_block(i, rb, utqs[i])
                # loss = ln(R) - s_pos, batched over all 24 columns
                nc.scalar.activation(
                    out=loss_sb[:], in_=rsum_sb[:],
                    func=mybir.ActivationFunctionType.Ln,
                )
                nc.vector.tensor_tensor(
                    loss_sb[:], loss_sb[:], spT_sb[:],
                    mybir.AluOpType.subtract,
                )

        nc.sync.dma_start(out=loss_d[:], in_=loss_sb[:])

    nc.compile()
    _split_multiwait_drains(nc)
    return nc


def prepare_inputs(z_seq, preds, neg_idx):
    """Host-side sharding/packing. Returns (in_maps, valid_counts)."""
    z_flat = np.asarray(z_seq, dtype=np.float32).reshape(BT, D)
    preds = np.asarray(preds, dtype=np.float32)
    neg_idx = np.asarray(neg_idx)

    norms = np.linalg.norm(z_flat, axis=1, keepdims=True)
    az = z_flat / np.maximum(norms, 1e-12)
    q4az = (AZ_SCALE * az).astype(ml_dtypes.float8_e4m3)  # quantize ONCE
    azq = np.ascontiguousarray(q4az.T.reshape(2, P, BT).transpose(1, 0, 2))

    # pt[d, i*4+kc*2+mc, e] = preds[i, mc*128+e, kc*128+d]
    pt = np.empty((P, H * 4, P), dtype=ml_dtypes.bfloat16)
    for i in range(H):
        w = preds[i]  # [e_out, d_in]
        for kc in range(2):
            for mc in range(2):
                blk = w[mc * P:(mc + 1) * P, kc * P:(kc + 1) * P]  # [e, d]
                pt[:, i * 4 + kc * 2 + mc, :] = blk.T.astype(ml_dtypes.bfloat16)

    # paired identity for the lncnt add (adds 2x the fp8 payload)
    idm = np.zeros((P, 2, P), dtype=ml_dtypes.float8_e4m3)
    eye = np.eye(P, dtype=np.float32).astype(ml_dtypes.float8_e4m3)
    idm[:, 0, :] = eye
    idm[:, 1, :] = eye

    in_maps = []
    valid_counts = np.zeros((N_CORES, H), dtype=np.int64)
    for c in range(N_CORES):
        n0 = c * NROW
        zat = np.zeros((P, H * 2, NROW), dtype=ml_dtypes.bfloat16)
        azpq = np.zeros((P, H * 2, NROW), dtype=ml_dtypes.float8_e4m3)
        lnc = np.zeros((P, NCOL, BT), dtype=ml_dtypes.float8_e4m3)
        for i, k in enumerate(HORIZONS):
            L = T - k
            BL = B * L
            nvalid = min(max(BL - n0, 0), NROW)
            valid_counts[c, i] = nvalid
            n = n0 + np.arange(NROW)
            nv = n[:nvalid]
            b = nv // L
            a_full = np.zeros(NROW, dtype=np.int64)
            a_full[:nvalid] = nv + b * k          # anchor flat rows
            p_full = np.zeros(NROW, dtype=np.int64)
            p_full[:nvalid] = nv + (b + 1) * k    # positive flat rows
            zat[:, i * 2:(i + 1) * 2, :] = (
                z_flat[a_full].T.reshape(2, P, NROW).transpose(1, 0, 2)
            ).astype(ml_dtypes.bfloat16)
            # positives gathered from the SAME quantized table as AZQ
            azpq[:, i * 2:(i + 1) * 2, :] = (
                q4az[p_full].T.reshape(2, P, NROW).transpose(1, 0, 2)
            )

            # dense counts: negatives multiplicity + 1 at the positive
            cm = np.zeros((NROW, BT), dtype=np.float32)
            rows = np.repeat(np.arange(nvalid), N_NEG)
            np.add.at(cm, (rows, neg_idx[i, nv, :].reshape(-1)), 1.0)
            cm[np.arange(NROW), p_full] += 1.0
            if nvalid < NROW:
                # pad rows: keep a single count so R>0 (host ignores them)
                cm[nvalid:] = 0.0
                cm[nvalid:, 0] = 1.0
            # per-row ||u|| with the same anchor gathering as the device
            u = z_flat[a_full].astype(np.float32) @ preds[i].T
            unorm = np.linalg.norm(u, axis=1)  # [NROW]
            half_scale = (2.0 * TAU) * unorm  # lnc * rs must equal ln(cnt)/2
            with np.errstate(divide="ignore"):
                lmat = np.where(
                    cm > 0.0,
                    np.log(np.maximum(cm, 1e-30)) * half_scale[:, None],
                    np.float32(LNC_NEG),
                )
            lq = lmat.astype(ml_dtypes.float8_e4m3)
            for rb in range(NBLK):
                lnc[:, i * NBLK + rb, :] = lq[rb * P:(rb + 1) * P]

        in_maps.append(
            {"azq": azq, "zat": zat, "azpq": azpq, "pt": pt, "idm": idm,
             "lnc": lnc}
        )
    return in_maps, valid_counts


def reduce_outputs(results, valid_counts):
    raw_w = {k: 1.0 / math.sqrt(k) for k in HORIZONS}
    tot_w = sum(raw_w.values())
    dve_corr = np.zeros(NCOL, dtype=np.float64)
    for col in DVE_COLS:
        dve_corr[col] = LN_PWL_BIAS
    total = np.float64(0.0)
    for i, k in enumerate(HORIZONS):
        L = T - k
        BL = B * L
        s = np.float64(0.0)
        for c in range(N_CORES):
            nvalid = int(valid_counts[c, i])
            if nvalid == 0:
                continue
            lm = results[c]["loss"].astype(np.float64)  # [P, NCOL]
            lm = lm - dve_corr[None, :]
            per_row = lm[:, i * NBLK:(i + 1) * NBLK].T.reshape(NROW)
            s += per_row[:nvalid].sum()
        total += (raw_w[k] / tot_w) * (s / BL)
    return np.float32(total)


_CACHED_NC = None


def kernel(z_seq, preds, neg_idx):
    global _CACHED_NC
    if _CACHED_NC is None:
        _CACHED_NC = build_program()
    nc = _CACHED_NC
    in_maps, valid_counts = prepare_inputs(z_seq, preds, neg_idx)
    res = run_bass_kernel_spmd(nc, in_maps, list(range(N_CORES)))
    return reduce_outputs(res.results, valid_counts)


if __name__ == "__main__":
    rng = np.random.default_rng(0)
    z = rng.standard_normal((B, T, D), dtype=np.float32)
    pr = (rng.standard_normal((H, D, D), dtype=np.float32) / np.sqrt(D)).astype(
        np.float32
    )
    ni = rng.integers(0, BT, size=(H, BT, N_NEG), dtype=np.int64)
    print(kernel(z, pr, ni))


# revision 17
# speedup vs baseline: 1.0005x; 1.0005x over previous
"""CPC InfoNCE loss kernel for Trainium2 (8 NeuronCores, data-parallel rows).

v4: fp8 DoubleRow similarity matmuls with the count-mask folded into PSUM,
exp split across ACT and DVE (Schraudolph bit-trick), no dense count DMA.

Per core (rows sharded across cores, 3 horizons x 8 blocks of 128 rows):
  - Host normalizes the pool table az = normalize(z_seq.reshape(BT, D)),
    quantizes 4*az to fp8-e4m3 once (AZQ, transposed), gathers the positives
    from the same quantized table (AZPQ), and packs predictor weights (bf16).
    Instead of dense bf16 counts it sends LNC [row, pool] fp8 with
    LNC = ln(multiplicity) * 2*tau*||u_row||  at sampled pool entries
    (positive included) and -240 elsewhere.
  - PE computes U^T = W @ Z_anchor^T in bf16 into PSUM; DVE casts to fp8.
    Norms ||u||^2 / positive logits via fp8 squares + ones-matmuls (PSUM
    accumulated), rs = 1/(4*tau*||u||) via DVE reciprocal + ACT sqrt.
  - Per 128-row block and 512-col slice PE issues TWO fp8 DoubleRow matmuls
    into one PSUM group: S = U_blk @ AZQ^T (K=256) plus IDM @ LNC (paired
    identity, stride-0 rhs) which adds 2*LNC. After the per-row exp scale
    rs, the PSUM value is logit + ln(cnt) (or ~-27 at unsampled entries, so
    exp gives 0 and the row sum needs no mask).
  - exp+reduce: ACT blocks use activation(Exp, scale=rs, accum_out) giving
    sum_j cnt_j e^j directly; Schraudolph blocks compute a piecewise-linear
    2^t on DVE (tensor_scalar f32->uint16 with scale rs*128*log2(e), bitcast
    to bf16) and reduce with a 4x-mode tensor_scalar accumulate. The exact
    E[(1+f)2^-f] = 0.5/ln(2)^2 bias of the PWL exp is subtracted on the host
    (ln R shifts by 0.0398844) for those blocks.
  - loss = ln(R) - s_pos per row; host averages with horizon weights.
"""

import sys

sys.path.insert(0, "/opt/trn_rl_repo")

import math
import os

import ml_dtypes
import numpy as np

import concourse.bass as bass
import concourse.tile as tile
from concourse import bacc
from concourse import mybir
from concourse.bass_utils import run_bass_kernel_spmd

# Problem constants (hardcoded per contract)
B, T, D = 16, 512, 256
BT = B * T  # 8192 pool entries
HORIZONS = (1, 5, 21)
H = len(HORIZONS)
N_NEG = 128
TAU = 0.07
N_CORES = 8

P = 128
NROW = 1024  # padded rows per core per horizon
NBLK = NROW // P  # 8
NCOL = H * NBLK  # 24 row-blocks per core
QCOL = 2048  # exp tile width (4 PSUM banks)
NQ = BT // QCOL  # 4
QCOL2 = 1024  # split-stream exp tile width (2 PSUM banks)
NQ2 = BT // QCOL2  # 8

AZ_SCALE = 4.0  # fp8 pre-scale on the pool table
# "masked" value: adds 2*(-144) to PSUM -> exp arg ~ -65 +- 12, so exp -> 0
# while the Schraudolph uint16 bits stay positive (arg > -88).
LNC_NEG = -144.0
LOG2E_128 = 128 * 1.4426950408889634  # Schraudolph scale
SCHRAUDOLPH_C = 16256.5  # 127<<7 | +0.5 for trunc->round
LN_PWL_BIAS = float(np.log(0.5 / np.log(2.0) ** 2))  # 0.0398844...

BF16 = mybir.dt.bfloat16
F32 = mybir.dt.float32
FP8 = mybir.dt.float8e4
U16 = mybir.dt.uint16

# block columns whose exp runs on DVE via the Schraudolph trick
N_DVE_EXP = int(os.environ.get("KERNEL_DVE_EXP", "9"))
_RB_PATTERNS = {
    0: (), 1: (5,), 2: (2, 6), 3: (2, 5, 7), 4: (1, 3, 5, 7),
    5: (1, 3, 5, 6, 7), 6: (1, 2, 3, 5, 6, 7), 7: (1, 2, 3, 4, 5, 6, 7),
    8: tuple(range(8)),
}


def _dve_exp_cols():
    base, rem = divmod(N_DVE_EXP, H)
    cols = set()
    for i in range(H):
        cnt = base + (1 if i < rem else 0)
        for rb in _RB_PATTERNS[min(cnt, 8)]:
            cols.add(i * NBLK + rb)
    return frozenset(cols)


DVE_COLS = _dve_exp_cols()


def _split_multiwait_drains(nc):
    """This walrus build accepts only one sync-wait command per TPB_CTRL
    instruction; TileContext's exit drain carries one wait per live proc.
    Split the extras into preceding single-wait drains."""
    for f in nc.m.functions:
        for bb in f.blocks:
            new_list = []
            for inst in bb.instructions:
                si = inst.sync_info
                if si is not None and si.on_wait and len(si.on_wait) > 1:
                    waits = list(si.on_wait)
                    for j, w in enumerate(waits[:-1]):
                        d = mybir.InstDrain(
                            name=f"{inst.name}-w{j}", ins=[], outs=[]
                        )
                        d.engine = inst.engine
                        d.sync_info = mybir.SyncInfo(on_wait=[w], on_update=[])
                        nc.register_instruction(d)
                        new_list.append(d)
                    si.on_wait = [waits[-1]]
                    inst.sync_info = si
                new_list.append(inst)
            bb.instructions[:] = new_list


def build_program(reps=1):
    reps = int(os.environ.get("KERNEL_REPS", reps))
    nc = bacc.Bacc(
        "TRN2", target_bir_lowering=False, debug=False, num_devices=N_CORES
    )

    azq_d = nc.declare_dram_parameter("azq", [P, 2, BT], FP8, isOutput=False)
    zat_d = nc.declare_dram_parameter("zat", [P, H * 2, NROW], BF16, isOutput=False)
    azpq_d = nc.declare_dram_parameter("azpq", [P, H * 2, NROW], FP8, isOutput=False)
    pt_d = nc.declare_dram_parameter("pt", [P, H * 4, P], BF16, isOutput=False)
    idm_d = nc.declare_dram_parameter("idm", [P, 2, P], FP8, isOutput=False)
    lnc_d = nc.declare_dram_parameter("lnc", [P, NCOL, BT], FP8, isOutput=False)
    loss_d = nc.declare_dram_parameter("loss", [P, NCOL], F32, isOutput=True)

    from contextlib import ExitStack, nullcontext

    MULT = mybir.AluOpType.mult
    ADD = mybir.AluOpType.add

    with tile.TileContext(nc) as tc, ExitStack() as ctx:
        singles = ctx.enter_context(tc.tile_pool(name="singles", bufs=1))
        utq_pool = ctx.enter_context(tc.tile_pool(name="utq", bufs=4))
        junk_pool = ctx.enter_context(tc.tile_pool(name="junk", bufs=2))
        c_pool = ctx.enter_context(tc.tile_pool(name="c", bufs=3))
        e_pool = ctx.enter_context(tc.tile_pool(name="e", bufs=2))
        racc_pool = ctx.enter_context(tc.tile_pool(name="racc", bufs=2))
        small = ctx.enter_context(tc.tile_pool(name="small", bufs=2))
        # two independent 2-bank PSUM streams so ACT-exp blocks and
        # DVE-Schraudolph blocks pipeline concurrently
        psum_a = ctx.enter_context(tc.tile_pool(name="psum_a", bufs=2, space="PSUM"))
        psum_d = ctx.enter_context(tc.tile_pool(name="psum_d", bufs=2, space="PSUM"))

        # ---- preload constants -------------------------------------------
        azq_sb = singles.tile([P, 2, BT], FP8)
        nc.sync.dma_start(out=azq_sb[:], in_=azq_d[:])
        zat_sb = singles.tile([P, H * 2, NROW], BF16)
        nc.sync.dma_start(out=zat_sb[:], in_=zat_d[:])
        azpq_sb = singles.tile([P, H * 2, NROW], FP8)
        nc.sync.dma_start(out=azpq_sb[:], in_=azpq_d[:])
        pt_sb = singles.tile([P, H * 4, P], BF16)
        nc.sync.dma_start(out=pt_sb[:], in_=pt_d[:])
        idm_sb = singles.tile([P, 2, P], FP8)
        nc.sync.dma_start(out=idm_sb[:], in_=idm_d[:])

        ones_sb = singles.tile([P, 1], BF16)
        nc.vector.memset(ones_sb[:], 1.0)
        one1_sb = singles.tile([1, 1], F32)
        nc.vector.memset(one1_sb[:], 1.0)

        loss_sb = singles.tile([P, NCOL], F32)
        rsum_sb = singles.tile([P, NCOL], F32)
        rsT_sb = singles.tile([P, NCOL], F32)
        rsT2_sb = singles.tile([P, NCOL], F32)  # rs * 128*log2(e)
        spT_sb = singles.tile([P, NCOL], F32)
        edump_sb = singles.tile([P, QCOL], BF16)  # ACT exp dummy out

        def emit_u_phase(i):
            """U^T = W @ Z_anchor^T (bf16 -> PSUM -> fp8), norms, positive
            logits, and the transposed per-block scale/pos columns."""
            utq = utq_pool.tile([P, 2, NROW], FP8, tag="utq")
            t_n = psum_d.tile([P, 1024], F32, tag="pd")
            t_p = psum_d.tile([P, 1024], F32, tag="pd")
            for mc in range(2):
                t_u = psum_a.tile([P, 1024], F32, tag="pa")
                for nh in range(2):
                    nsl = slice(nh * 512, (nh + 1) * 512)
                    pu = t_u[:, nsl]
                    for kc in range(2):
                        nc.tensor.matmul(
                            pu,
                            pt_sb[:, i * 4 + kc * 2 + mc, :],
                            zat_sb[:, i * 2 + kc, nsl],
                            start=(kc == 0),
                            stop=(kc == 1),
                        )
                    nc.vector.tensor_copy(out=utq[:, mc, nsl], in_=pu)
                # norms and positive logits from the fp8-rounded values
                usq = junk_pool.tile([P, NROW], BF16, tag="usq")
                nc.vector.tensor_tensor(
                    usq[:], utq[:, mc, :], utq[:, mc, :], MULT
                )
                upr = junk_pool.tile([P, NROW], BF16, tag="upr")
                nc.vector.tensor_tensor(
                    upr[:], utq[:, mc, :], azpq_sb[:, i * 2 + mc, :], MULT
                )
                # column sums via ones-matmuls, accumulated in PSUM
                for nh in range(2):
                    nsl = slice(nh * 512, (nh + 1) * 512)
                    nc.tensor.matmul(
                        t_n[0:1, nsl], ones_sb[:], usq[:, nsl],
                        start=(mc == 0), stop=(mc == 1),
                    )
                    nc.tensor.matmul(
                        t_p[0:1, nsl], ones_sb[:], upr[:, nsl],
                        start=(mc == 0), stop=(mc == 1),
                    )
            # rs = 1/(4*tau*||u||) = sqrt(1/(16 tau^2 ||u||^2))
            recip = small.tile([1, NROW], F32, tag="recip")
            nc.vector.reciprocal(out=recip[:], in_=t_n[0:1, :])
            rs_flat = small.tile([1, NROW], F32, tag="rsflat")
            nc.scalar.activation(
                out=rs_flat[:], in_=recip[:],
                func=mybir.ActivationFunctionType.Sqrt,
                scale=float(1.0 / (16.0 * TAU * TAU)),
            )
            # sp = raw4_pos_dot * rs  (the positive logit)
            sp_flat = small.tile([1, NROW], F32, tag="spflat")
            nc.vector.tensor_tensor(
                sp_flat[:], t_p[0:1, :], rs_flat[:], MULT
            )
            # transpose the per-row scalars into per-block columns
            t_t = psum_a.tile([P, 1024], F32, tag="pa")
            for rb in range(NBLK):
                nc.tensor.matmul(
                    t_t[:, rb:rb + 1],
                    rs_flat[0:1, rb * P:(rb + 1) * P],
                    one1_sb[:], start=True, stop=True,
                )
                nc.tensor.matmul(
                    t_t[:, NBLK + rb:NBLK + rb + 1],
                    sp_flat[0:1, rb * P:(rb + 1) * P],
                    one1_sb[:], start=True, stop=True,
                )
            csl = slice(i * NBLK, (i + 1) * NBLK)
            nc.vector.tensor_copy(out=rsT_sb[:, csl], in_=t_t[:, 0:NBLK])
            nc.vector.tensor_copy(
                out=spT_sb[:, csl], in_=t_t[:, NBLK:2 * NBLK]
            )
            nc.vector.tensor_scalar(
                out=rsT2_sb[:, csl], in0=rsT_sb[:, csl],
                scalar1=float(LOG2E_128), scalar2=None, op0=MULT,
            )
            return utq

        def emit_s_block(i, rb, utq):
            """fp8 DR S matmuls + lncnt add -> exp -> row-sum."""
            col = i * NBLK + rb
            lnc_sb = c_pool.tile([P, 1, BT], FP8, tag="c")
            nc.sync.dma_start(out=lnc_sb[:], in_=lnc_d[:, col:col + 1, :])
            on_dve = col in DVE_COLS
            if on_dve:
                eu = e_pool.tile([P, BT], U16, tag="e")
            else:
                racc = racc_pool.tile([P, NQ2], F32, tag="racc")
            for q in range(NQ2):
                if on_dve:
                    ps = psum_d.tile([P, QCOL2], F32, tag="pd")
                else:
                    ps = psum_a.tile([P, QCOL2], F32, tag="pa")
                no_add = int(os.environ.get("KERNEL_NO_LNCADD", "0"))
                mmcols = int(os.environ.get("KERNEL_MMCOLS", "512"))
                for sq in range(QCOL2 // mmcols):
                    c0 = q * QCOL2 + sq * mmcols
                    sl = ps[:, sq * mmcols:(sq + 1) * mmcols]
                    nc.tensor.matmul(
                        sl,
                        utq[:, 0:2, rb * P:(rb + 1) * P],
                        azq_sb[:, 0:2, c0:c0 + mmcols],
                        start=True, stop=no_add == 1,
                        perf_mode=mybir.MatmulPerfMode.DoubleRow,
                    )
                    if not no_add:
                        nc.tensor.matmul(
                            sl,
                            idm_sb[:],
                            lnc_sb[:, 0:1, c0:c0 + mmcols].broadcast_to(
                                [P, 2, mmcols]
                            ),
                            start=False, stop=True,
                            perf_mode=mybir.MatmulPerfMode.DoubleRow,
                        )
                if on_dve:
                    # e~ = 2^(x*log2e) via PWL bit-trick: uint16 bits of bf16
                    nc.vector.tensor_scalar(
                        out=eu[:, q * QCOL2:(q + 1) * QCOL2], in0=ps[:],
                        scalar1=rsT2_sb[:, col:col + 1],
                        scalar2=float(SCHRAUDOLPH_C),
                        op0=MULT, op1=ADD,
                    )
                else:
                    nc.scalar.activation(
                        out=edump_sb[:, 0:QCOL2], in_=ps[:],
                        func=mybir.ActivationFunctionType.Exp,
                        scale=rsT_sb[:, col:col + 1],
                        accum_out=racc[:, q:q + 1],
                    )
            if on_dve:
                ebf = eu[:].bitcast(BF16)
                nc.vector.tensor_scalar(
                    out=ebf, in0=ebf, scalar1=1.0, scalar2=0.0,
                    op0=MULT, op1=ADD,
                    accum_out=rsum_sb[:, col:col + 1],
                )
            else:
                nc.vector.tensor_scalar(
                    out=racc[:], in0=racc[:], scalar1=1.0, scalar2=0.0,
                    op0=MULT, op1=ADD,
                    accum_out=rsum_sb[:, col:col + 1],
                )

        unroll = int(os.environ.get("KERNEL_UNROLL", "1"))
        jit_u = int(os.environ.get("KERNEL_JIT_U", "0"))
        loop_cm = tc.For_i(0, reps, 1) if reps > 1 else nullcontext()
        with loop_cm:
            for _ in range(unroll):
                if jit_u:
                    # just-in-time U phases: emit U(i+1) two blocks into
                    # horizon i's S stream so the exp pipeline never drains
                    utqs = [emit_u_phase(0), None, None]
                    for i in range(H):
                        for rb in range(NBLK):
                            emit_s_block(i, rb, utqs[i])
                            if rb == 1 and i + 1 < H:
                                utqs[i + 1] = emit_u_phase(i + 1)
                else:
                    utqs = [emit_u_phase(i) for i in range(H)]
                    for i in range(H):
                        for rb in range(NBLK):
                            emit_s_block(i, rb, utqs[i])
                # loss = ln(R) - s_pos, batched over all 24 columns
                nc.scalar.activation(
                    out=loss_sb[:], in_=rsum_sb[:],
                    func=mybir.ActivationFunctionType.Ln,
                )
                nc.vector.tensor_tensor(
                    loss_sb[:], loss_sb[:], spT_sb[:],
                    mybir.AluOpType.subtract,
                )

        nc.sync.dma_start(out=loss_d[:], in_=loss_sb[:])

    nc.compile()
    _split_multiwait_drains(nc)
    return nc


def prepare_inputs(z_seq, preds, neg_idx):
    """Host-side sharding/packing. Returns (in_maps, valid_counts)."""
    z_flat = np.asarray(z_seq, dtype=np.float32).reshape(BT, D)
    preds = np.asarray(preds, dtype=np.float32)
    neg_idx = np.asarray(neg_idx)

    norms = np.linalg.norm(z_flat, axis=1, keepdims=True)
    az = z_flat / np.maximum(norms, 1e-12)
    q4az = (AZ_SCALE * az).astype(ml_dtypes.float8_e4m3)  # quantize ONCE
    azq = np.ascontiguousarray(q4az.T.reshape(2, P, BT).transpose(1, 0, 2))

    # pt[d, i*4+kc*2+mc, e] = preds[i, mc*128+e, kc*128+d]
    pt = np.empty((P, H * 4, P), dtype=ml_dtypes.bfloat16)
    for i in range(H):
        w = preds[i]  # [e_out, d_in]
        for kc in range(2):
            for mc in range(2):
                blk = w[mc * P:(mc + 1) * P, kc * P:(kc + 1) * P]  # [e, d]
                pt[:, i * 4 + kc * 2 + mc, :] = blk.T.astype(ml_dtypes.bfloat16)

    # paired identity for the lncnt add (adds 2x the fp8 payload)
    idm = np.zeros((P, 2, P), dtype=ml_dtypes.float8_e4m3)
    eye = np.eye(P, dtype=np.float32).astype(ml_dtypes.float8_e4m3)
    idm[:, 0, :] = eye
    idm[:, 1, :] = eye

    in_maps = []
    valid_counts = np.zeros((N_CORES, H), dtype=np.int64)
    for c in range(N_CORES):
        n0 = c * NROW
        zat = np.zeros((P, H * 2, NROW), dtype=ml_dtypes.bfloat16)
        azpq = np.zeros((P, H * 2, NROW), dtype=ml_dtypes.float8_e4m3)
        lnc = np.zeros((P, NCOL, BT), dtype=ml_dtypes.float8_e4m3)
        for i, k in enumerate(HORIZONS):
            L = T - k
            BL = B * L
            nvalid = min(max(BL - n0, 0), NROW)
            valid_counts[c, i] = nvalid
            n = n0 + np.arange(NROW)
            nv = n[:nvalid]
            b = nv // L
            a_full = np.zeros(NROW, dtype=np.int64)
            a_full[:nvalid] = nv + b * k          # anchor flat rows
            p_full = np.zeros(NROW, dtype=np.int64)
            p_full[:nvalid] = nv + (b + 1) * k    # positive flat rows
            zat[:, i * 2:(i + 1) * 2, :] = (
                z_flat[a_full].T.reshape(2, P, NROW).transpose(1, 0, 2)
            ).astype(ml_dtypes.bfloat16)
            # positives gathered from the SAME quantized table as AZQ
            azpq[:, i * 2:(i + 1) * 2, :] = (
                q4az[p_full].T.reshape(2, P, NROW).transpose(1, 0, 2)
            )

            # dense counts: negatives multiplicity + 1 at the positive
            cm = np.zeros((NROW, BT), dtype=np.float32)
            rows = np.repeat(np.arange(nvalid), N_NEG)
            np.add.at(cm, (rows, neg_idx[i, nv, :].reshape(-1)), 1.0)
            cm[np.arange(NROW), p_full] += 1.0
            if nvalid < NROW:
                # pad rows: keep a single count so R>0 (host ignores them)
                cm[nvalid:] = 0.0
                cm[nvalid:, 0] = 1.0
            # per-row ||u|| with the same anchor gathering as the device
            u = z_flat[a_full].astype(np.float32) @ preds[i].T
            unorm = np.linalg.norm(u, axis=1)  # [NROW]
            half_scale = (2.0 * TAU) * unorm  # lnc * rs must equal ln(cnt)/2
            with np.errstate(divide="ignore"):
                lmat = np.where(
                    cm > 0.0,
                    np.log(np.maximum(cm, 1e-30)) * half_scale[:, None],
                    np.float32(LNC_NEG),
                )
            lq = lmat.astype(ml_dtypes.float8_e4m3)
            for rb in range(NBLK):
                lnc[:, i * NBLK + rb, :] = lq[rb * P:(rb + 1) * P]

        in_maps.append(
            {"azq": azq, "zat": zat, "azpq": azpq, "pt": pt, "idm": idm,
             "lnc": lnc}
        )
    return in_maps, valid_counts


def reduce_outputs(results, valid_counts):
    raw_w = {k: 1.0 / math.sqrt(k) for k in HORIZONS}
    tot_w = sum(raw_w.values())
    dve_corr = np.zeros(NCOL, dtype=np.float64)
    for col in DVE_COLS:
        dve_corr[col] = LN_PWL_BIAS
    total = np.float64(0.0)
    for i, k in enumerate(HORIZONS):
        L = T - k
        BL = B * L
        s = np.float64(0.0)
        for c in range(N_CORES):
            nvalid = int(valid_counts[c, i])
            if nvalid == 0:
                continue
            lm = results[c]["loss"].astype(np.float64)  # [P, NCOL]
            lm = lm - dve_corr[None, :]
            per_row = lm[:, i * NBLK:(i + 1) * NBLK].T.reshape(NROW)
            s += per_row[:nvalid].sum()
        total += (raw_w[k] / tot_w) * (s / BL)
    return np.float32(total)


_CACHED_NC = None


def kernel(z_seq, preds, neg_idx):
    global _CACHED_NC
    if _CACHED_NC is None:
        _CACHED_NC = build_program()
    nc = _CACHED_NC
    in_maps, valid_counts = prepare_inputs(z_seq, preds, neg_idx)
    res = run_bass_kernel_spmd(nc, in_maps, list(range(N_CORES)))
    return reduce_outputs(res.results, valid_counts)


if __name__ == "__main__":
    rng = np.random.default_rng(0)
    z = rng.standard_normal((B, T, D), dtype=np.float32)
    pr = (rng.standard_normal((H, D, D), dtype=np.float32) / np.sqrt(D)).astype(
        np.float32
    )
    ni = rng.integers(0, BT, size=(H, BT, N_NEG), dtype=np.int64)
    print(kernel(z, pr, ni))
